# revision 10
# baseline (speedup 1.0000x reference)
"""Trainium2 Bass kernel: inclusive cumsum along L for X (4, 8192, 32, 32) f32.

Strategy (8 NeuronCores, SPMD):
  - View X as (B=4, L=8192, C=1024) with C = D*N flattened. The 4096 scan
    rows (b, c) are independent; shard them 8 ways: core i gets b = i//2 and
    the c-half h = i%2, i.e. a (8192, 512) slab whose DRAM rows are 2 KiB
    contiguous.
  - Per core: stream 512-long L superblocks (1 MiB batched DMAs). Each
    (128 l, 128 c) sub-tile is PE-transposed into PSUM banks laid out as
    (128 c, 512 l). The VectorE tensor_tensor_scan runs the cumsum along the
    free (l) dim, chaining superblocks via the `initial` operand (carry =
    last column of the previous scan output). Scan outputs are PE-transposed
    back to natural (l, c) layout in PSUM, copied to SBUF by ScalarE, and
    DMA'd out as 1 MiB transfers.
  - Engine budget per core: DMA ~100 us (bottleneck), DVE scans ~78 us,
    PE transposes ~60-90 us, ACT copies ~40 us. In-DMAs issue on the Sync
    HWDGE ring, out-DMAs on GPSIMD SWDGE for queue diversity.
"""

import numpy as np
from contextlib import ExitStack

import concourse.bass as bass
import concourse.tile as tile
from concourse import bacc, masks, mybir
from concourse.bass_utils import run_bass_kernel_spmd

N_CORES = 8
B, L, D, N = 4, 8192, 32, 32
C_FULL = D * N          # 1024 columns per batch entry
C = C_FULL // 2         # 512 columns per core
P = 128                 # partitions
SUPER = 512             # L elems per superblock
N_SUPER = L // SUPER    # 16
BLKS = SUPER // P       # 4 L-blocks per superblock
CGRP = C // P           # 4 column groups per core

_CACHE = {}


def _build_program():
    f32 = mybir.dt.float32
    nc = bacc.Bacc(
        trn_type="TRN2", debug=False, num_devices=N_CORES, num_swdge_queues=2
    )
    x = nc.dram_tensor("x", [L, C], f32, kind="ExternalInput").ap()
    y = nc.dram_tensor("y", [L, C], f32, kind="ExternalOutput").ap()

    with tile.TileContext(nc) as tc, ExitStack() as ctx:
        const_pool = ctx.enter_context(tc.tile_pool(name="const", bufs=1))
        xin_pool = ctx.enter_context(tc.tile_pool(name="xin", bufs=6))
        scano_pool = ctx.enter_context(tc.tile_pool(name="scano", bufs=2))
        yout_pool = ctx.enter_context(tc.tile_pool(name="yout", bufs=6))
        inps_pool = ctx.enter_context(tc.tile_pool(name="inps", bufs=4, space="PSUM"))
        outps_pool = ctx.enter_context(tc.tile_pool(name="outps", bufs=2, space="PSUM"))

        ident = const_pool.tile([P, P], f32, name="ident")
        masks.make_identity(nc, ident[:])
        zeros = const_pool.tile([P, SUPER], f32, name="zeros")
        nc.gpsimd.memset(zeros[:], 0.0)

        prev = [None] * CGRP
        for t in range(N_SUPER):
            # ---- load the whole superblock with one 1 MiB DMA ----
            # DRAM rows l = t*512 + ks*128 + p; element order [p][ks][c] on
            # both sides so the 3D APs pair up.
            xt = xin_pool.tile([P, BLKS * C], f32, name="xt", tag="xt", bufs=6)
            in_eng = nc.sync if t % 2 == 0 else nc.gpsimd
            for half in range(2):
                lo = t * SUPER + half * (SUPER // 2)
                src = x[lo : lo + SUPER // 2, :].rearrange(
                    "(ks p) c -> p ks c", p=P
                )
                dst = xt[:, half * 2 * C : (half + 1) * 2 * C].rearrange(
                    "p (ks c) -> p ks c", ks=2
                )
                in_eng.dma_start(out=dst, in_=src)

            # ---- transpose into (c, l) PSUM banks; scan along l ----
            souts = []
            for j in range(CGRP):
                ib = inps_pool.tile([P, SUPER], f32, name="ib", tag="ib", bufs=4)
                for ks in range(BLKS):
                    nc.tensor.transpose(
                        ib[:, ks * P : (ks + 1) * P],
                        xt[:, ks * C + j * P : ks * C + (j + 1) * P],
                        ident[:],
                    )
                so = scano_pool.tile(
                    [P, SUPER], f32, name=f"so{j}", tag=f"so{j}", bufs=3
                )
                init = 0.0 if t == 0 else prev[j][:, SUPER - 1 : SUPER]
                nc.vector.tensor_tensor_scan(
                    so[:], ib[:], zeros[:], init,
                    mybir.AluOpType.add, mybir.AluOpType.add,
                )
                souts.append(so)
            prev = souts

            # ---- transpose back to (l, c); ScalarE copies PSUM->SBUF ----
            yt = yout_pool.tile([P, BLKS * C], f32, name="yt", tag="yt", bufs=6)
            for half in range(2):
                ob = outps_pool.tile([P, 2 * C], f32, name="ob", tag="ob", bufs=2)
                for i2 in range(2):
                    i = half * 2 + i2
                    for j in range(CGRP):
                        nc.tensor.transpose(
                            ob[:, i2 * C + j * P : i2 * C + (j + 1) * P],
                            souts[j][:, i * P : (i + 1) * P],
                            ident[:],
                        )
                nc.scalar.copy(yt[:, half * 2 * C : (half + 1) * 2 * C], ob[:])

            ydst = y[t * SUPER : (t + 1) * SUPER, :].rearrange(
                "(ks p) c -> p ks c", p=P
            )
            ysrc = yt[:].rearrange("p (ks c) -> p ks c", ks=BLKS)
            out_eng = nc.gpsimd if t % 2 == 0 else nc.sync
            out_eng.dma_start(out=ydst, in_=ysrc)

    nc.compile()
    return nc


def _get_program():
    if "nc" not in _CACHE:
        _CACHE["nc"] = _build_program()
    return _CACHE["nc"]


def _shard(X):
    """(4, 8192, 32, 32) -> 8 contiguous (8192, 512) slabs."""
    Xv = X.reshape(B, L, C_FULL)
    shards = []
    for i in range(N_CORES):
        b, h = i // 2, i % 2
        shards.append(np.ascontiguousarray(Xv[b, :, h * C : (h + 1) * C]))
    return shards


def _unshard(parts):
    out = np.empty((B, L, C_FULL), dtype=np.float32)
    for i in range(N_CORES):
        b, h = i // 2, i % 2
        out[b, :, h * C : (h + 1) * C] = parts[i]
    return out.reshape(B, L, D, N)


def kernel(X_in, _trace=False, _tmpdir=None, _trace_cores=None):
    X = np.asarray(X_in, dtype=np.float32)
    assert X.shape == (B, L, D, N), X.shape
    nc = _get_program()
    in_maps = [{"x": s} for s in _shard(X)]
    kwargs = {}
    if _trace:
        kwargs = dict(
            trace=True,
            tmpdir=_tmpdir,
            trace_cores=_trace_cores or list(range(N_CORES)),
        )
    res = run_bass_kernel_spmd(nc, in_maps, core_ids=list(range(N_CORES)), **kwargs)
    out = _unshard([res.results[i]["y"] for i in range(N_CORES)])
    kernel.last_results = res
    return out


# revision 11
# speedup vs baseline: 1.0081x; 1.0081x over previous
"""Trainium2 Bass kernel: inclusive cumsum along L for X (4, 8192, 32, 32) f32.

Strategy (8 NeuronCores, SPMD):
  - View X as (B=4, L=8192, C=1024) with C = D*N flattened. The 4096 scan
    rows (b, c) are independent; shard them 8 ways: core i gets b = i//2 and
    the c-half h = i%2, i.e. a (8192, 512) slab whose DRAM rows are 2 KiB
    contiguous.
  - Per core: stream 512-long L superblocks (1 MiB batched DMAs). Each
    (128 l, 128 c) sub-tile is PE-transposed into PSUM banks laid out as
    (128 c, 512 l). The VectorE tensor_tensor_scan runs the cumsum along the
    free (l) dim, chaining superblocks via the `initial` operand (carry =
    last column of the previous scan output). Scan outputs are PE-transposed
    back to natural (l, c) layout in PSUM, copied to SBUF by ScalarE, and
    DMA'd out as 1 MiB transfers.
  - Engine budget per core: DMA ~100 us (bottleneck), DVE scans ~78 us,
    PE transposes ~60-90 us, ACT copies ~40 us. In-DMAs issue on the Sync
    HWDGE ring, out-DMAs on GPSIMD SWDGE for queue diversity.
"""

import numpy as np
from contextlib import ExitStack

import concourse.bass as bass
import concourse.tile as tile
from concourse import bacc, masks, mybir
from concourse.bass_utils import run_bass_kernel_spmd

N_CORES = 8
B, L, D, N = 4, 8192, 32, 32
C_FULL = D * N          # 1024 columns per batch entry
C = C_FULL // 2         # 512 columns per core
P = 128                 # partitions
SUPER = 512             # L elems per superblock
N_SUPER = L // SUPER    # 16
BLKS = SUPER // P       # 4 L-blocks per superblock
CGRP = C // P           # 4 column groups per core

_CACHE = {}


def _build_program():
    f32 = mybir.dt.float32
    nc = bacc.Bacc(
        trn_type="TRN2", debug=False, num_devices=N_CORES, num_swdge_queues=2
    )
    x = nc.dram_tensor("x", [L, C], f32, kind="ExternalInput").ap()
    y = nc.dram_tensor("y", [L, C], f32, kind="ExternalOutput").ap()

    with tile.TileContext(nc) as tc, ExitStack() as ctx:
        const_pool = ctx.enter_context(tc.tile_pool(name="const", bufs=1))
        xin_pool = ctx.enter_context(tc.tile_pool(name="xin", bufs=6))
        scano_pool = ctx.enter_context(tc.tile_pool(name="scano", bufs=2))
        yout_pool = ctx.enter_context(tc.tile_pool(name="yout", bufs=6))
        inps_pool = ctx.enter_context(tc.tile_pool(name="inps", bufs=4, space="PSUM"))
        outps_pool = ctx.enter_context(tc.tile_pool(name="outps", bufs=2, space="PSUM"))

        ident = const_pool.tile([P, P], f32, name="ident")
        masks.make_identity(nc, ident[:])
        zeros = const_pool.tile([P, SUPER], f32, name="zeros")
        nc.gpsimd.memset(zeros[:], 0.0)

        prev = [None] * CGRP
        for t in range(N_SUPER):
            # ---- load the whole superblock with one 1 MiB DMA ----
            # DRAM rows l = t*512 + ks*128 + p; element order [p][ks][c] on
            # both sides so the 3D APs pair up.
            xt = xin_pool.tile([P, BLKS * C], f32, name="xt", tag="xt", bufs=6)
            src = x[t * SUPER : (t + 1) * SUPER, :].rearrange(
                "(ks p) c -> p ks c", p=P
            )
            dst = xt[:].rearrange("p (ks c) -> p ks c", ks=BLKS)
            in_eng = nc.sync if t % 2 == 0 else nc.gpsimd
            in_eng.dma_start(out=dst, in_=src)

            # ---- transpose into (c, l) PSUM banks; scan along l ----
            souts = []
            for j in range(CGRP):
                ib = inps_pool.tile([P, SUPER], f32, name="ib", tag="ib", bufs=4)
                for ks in range(BLKS):
                    nc.tensor.transpose(
                        ib[:, ks * P : (ks + 1) * P],
                        xt[:, ks * C + j * P : ks * C + (j + 1) * P],
                        ident[:],
                    )
                so = scano_pool.tile(
                    [P, SUPER], f32, name=f"so{j}", tag=f"so{j}", bufs=2
                )
                init = 0.0 if t == 0 else prev[j][:, SUPER - 1 : SUPER]
                nc.vector.tensor_tensor_scan(
                    so[:], ib[:], zeros[:], init,
                    mybir.AluOpType.add, mybir.AluOpType.add,
                )
                souts.append(so)
            prev = souts

            # ---- transpose back to (l, c); ScalarE copies PSUM->SBUF ----
            yt = yout_pool.tile([P, BLKS * C], f32, name="yt", tag="yt", bufs=6)
            for half in range(2):
                ob = outps_pool.tile([P, 2 * C], f32, name="ob", tag="ob", bufs=2)
                for i2 in range(2):
                    i = half * 2 + i2
                    for j in range(CGRP):
                        nc.tensor.transpose(
                            ob[:, i2 * C + j * P : i2 * C + (j + 1) * P],
                            souts[j][:, i * P : (i + 1) * P],
                            ident[:],
                        )
                nc.scalar.copy(yt[:, half * 2 * C : (half + 1) * 2 * C], ob[:])

            ydst = y[t * SUPER : (t + 1) * SUPER, :].rearrange(
                "(ks p) c -> p ks c", p=P
            )
            ysrc = yt[:].rearrange("p (ks c) -> p ks c", ks=BLKS)
            out_eng = nc.gpsimd if t % 2 == 0 else nc.sync
            out_eng.dma_start(out=ydst, in_=ysrc)

    nc.compile()
    return nc


def _get_program():
    if "nc" not in _CACHE:
        _CACHE["nc"] = _build_program()
    return _CACHE["nc"]


def _shard(X):
    """(4, 8192, 32, 32) -> 8 contiguous (8192, 512) slabs."""
    Xv = X.reshape(B, L, C_FULL)
    shards = []
    for i in range(N_CORES):
        b, h = i // 2, i % 2
        shards.append(np.ascontiguousarray(Xv[b, :, h * C : (h + 1) * C]))
    return shards


def _unshard(parts):
    out = np.empty((B, L, C_FULL), dtype=np.float32)
    for i in range(N_CORES):
        b, h = i // 2, i % 2
        out[b, :, h * C : (h + 1) * C] = parts[i]
    return out.reshape(B, L, D, N)


def kernel(X_in, _trace=False, _tmpdir=None, _trace_cores=None):
    X = np.asarray(X_in, dtype=np.float32)
    assert X.shape == (B, L, D, N), X.shape
    nc = _get_program()
    in_maps = [{"x": s} for s in _shard(X)]
    kwargs = {}
    if _trace:
        kwargs = dict(
            trace=True,
            tmpdir=_tmpdir,
            trace_cores=_trace_cores or list(range(N_CORES)),
        )
    res = run_bass_kernel_spmd(nc, in_maps, core_ids=list(range(N_CORES)), **kwargs)
    out = _unshard([res.results[i]["y"] for i in range(N_CORES)])
    kernel.last_results = res
    return out


# revision 12
# speedup vs baseline: 1.0313x; 1.0230x over previous
"""Trainium2 Bass kernel: inclusive cumsum along L for X (4, 8192, 32, 32) f32.

Strategy (8 NeuronCores, SPMD):
  - View X as (B=4, L=8192, C=1024) with C = D*N flattened. The 4096 scan
    rows (b, c) are independent; shard them 8 ways: core i gets b = i//2 and
    the c-half h = i%2, i.e. a (8192, 512) slab whose DRAM rows are 2 KiB
    contiguous.
  - Per core: stream 512-long L superblocks (1 MiB batched DMAs). Each
    (128 l, 128 c) sub-tile is PE-transposed into PSUM banks laid out as
    (128 c, 512 l). The VectorE tensor_tensor_scan runs the cumsum along the
    free (l) dim, chaining superblocks via the `initial` operand (carry =
    last column of the previous scan output). Scan outputs are PE-transposed
    back to natural (l, c) layout in PSUM, copied to SBUF by ScalarE, and
    DMA'd out as 1 MiB transfers.
  - Engine budget per core: DMA ~100 us (bottleneck), DVE scans ~78 us,
    PE transposes ~60-90 us, ACT copies ~40 us. In-DMAs issue on the Sync
    HWDGE ring, out-DMAs on GPSIMD SWDGE for queue diversity.
"""

import numpy as np
from contextlib import ExitStack

import concourse.bass as bass
import concourse.tile as tile
from concourse import bacc, masks, mybir
from concourse.bass_utils import run_bass_kernel_spmd

N_CORES = 8
B, L, D, N = 4, 8192, 32, 32
C_FULL = D * N          # 1024 columns per batch entry
C = C_FULL // 2         # 512 columns per core
P = 128                 # partitions
SUPER = 512             # L elems per superblock
N_SUPER = L // SUPER    # 16
BLKS = SUPER // P       # 4 L-blocks per superblock
CGRP = C // P           # 4 column groups per core

_CACHE = {}


def _build_program():
    f32 = mybir.dt.float32
    nc = bacc.Bacc(
        trn_type="TRN2", debug=False, num_devices=N_CORES, num_swdge_queues=2
    )
    x = nc.dram_tensor("x", [L, C], f32, kind="ExternalInput").ap()
    y = nc.dram_tensor("y", [L, C], f32, kind="ExternalOutput").ap()

    with tile.TileContext(nc) as tc, ExitStack() as ctx:
        const_pool = ctx.enter_context(tc.tile_pool(name="const", bufs=1))
        xin_pool = ctx.enter_context(tc.tile_pool(name="xin", bufs=6))
        scano_pool = ctx.enter_context(tc.tile_pool(name="scano", bufs=2))
        yout_pool = ctx.enter_context(tc.tile_pool(name="yout", bufs=6))
        inps_pool = ctx.enter_context(tc.tile_pool(name="inps", bufs=4, space="PSUM"))
        outps_pool = ctx.enter_context(tc.tile_pool(name="outps", bufs=2, space="PSUM"))

        ident = const_pool.tile([P, P], f32, name="ident")
        masks.make_identity(nc, ident[:])
        zeros = const_pool.tile([P, SUPER], f32, name="zeros")
        nc.gpsimd.memset(zeros[:], 0.0)

        prev = [None] * CGRP
        for t in range(N_SUPER):
            # ---- load the whole superblock with one 1 MiB DMA ----
            # DRAM rows l = t*512 + ks*128 + p; element order [p][ks][c] on
            # both sides so the 3D APs pair up.
            xt = xin_pool.tile([P, BLKS * C], f32, name="xt", tag="xt", bufs=6)
            src = x[t * SUPER : (t + 1) * SUPER, :].rearrange(
                "(ks p) c -> p ks c", p=P
            )
            dst = xt[:].rearrange("p (ks c) -> p ks c", ks=BLKS)
            in_eng = (nc.sync, nc.gpsimd, nc.scalar)[t % 3]
            in_eng.dma_start(out=dst, in_=src)

            # ---- transpose into (c, l) PSUM banks; scan along l ----
            souts = []
            for j in range(CGRP):
                ib = inps_pool.tile([P, SUPER], f32, name="ib", tag="ib", bufs=4)
                for ks in range(BLKS):
                    nc.tensor.transpose(
                        ib[:, ks * P : (ks + 1) * P],
                        xt[:, ks * C + j * P : ks * C + (j + 1) * P],
                        ident[:],
                    )
                so = scano_pool.tile(
                    [P, SUPER], f32, name=f"so{j}", tag=f"so{j}", bufs=2
                )
                init = 0.0 if t == 0 else prev[j][:, SUPER - 1 : SUPER]
                nc.vector.tensor_tensor_scan(
                    so[:], ib[:], zeros[:], init,
                    mybir.AluOpType.add, mybir.AluOpType.add,
                )
                souts.append(so)
            prev = souts

            # ---- transpose back to (l, c); ScalarE copies PSUM->SBUF ----
            yt = yout_pool.tile([P, BLKS * C], f32, name="yt", tag="yt", bufs=6)
            for half in range(2):
                ob = outps_pool.tile([P, 2 * C], f32, name="ob", tag="ob", bufs=2)
                for i2 in range(2):
                    i = half * 2 + i2
                    for j in range(CGRP):
                        nc.tensor.transpose(
                            ob[:, i2 * C + j * P : i2 * C + (j + 1) * P],
                            souts[j][:, i * P : (i + 1) * P],
                            ident[:],
                        )
                nc.scalar.copy(yt[:, half * 2 * C : (half + 1) * 2 * C], ob[:])

            ydst = y[t * SUPER : (t + 1) * SUPER, :].rearrange(
                "(ks p) c -> p ks c", p=P
            )
            ysrc = yt[:].rearrange("p (ks c) -> p ks c", ks=BLKS)
            out_eng = nc.gpsimd if t % 2 == 0 else nc.sync
            out_eng.dma_start(out=ydst, in_=ysrc)

    nc.compile()
    return nc


def _get_program():
    if "nc" not in _CACHE:
        _CACHE["nc"] = _build_program()
    return _CACHE["nc"]


def _shard(X):
    """(4, 8192, 32, 32) -> 8 contiguous (8192, 512) slabs."""
    Xv = X.reshape(B, L, C_FULL)
    shards = []
    for i in range(N_CORES):
        b, h = i // 2, i % 2
        shards.append(np.ascontiguousarray(Xv[b, :, h * C : (h + 1) * C]))
    return shards


def _unshard(parts):
    out = np.empty((B, L, C_FULL), dtype=np.float32)
    for i in range(N_CORES):
        b, h = i // 2, i % 2
        out[b, :, h * C : (h + 1) * C] = parts[i]
    return out.reshape(B, L, D, N)


def kernel(X_in, _trace=False, _tmpdir=None, _trace_cores=None):
    X = np.asarray(X_in, dtype=np.float32)
    assert X.shape == (B, L, D, N), X.shape
    nc = _get_program()
    in_maps = [{"x": s} for s in _shard(X)]
    kwargs = {}
    if _trace:
        kwargs = dict(
            trace=True,
            tmpdir=_tmpdir,
            trace_cores=_trace_cores or list(range(N_CORES)),
        )
    res = run_bass_kernel_spmd(nc, in_maps, core_ids=list(range(N_CORES)), **kwargs)
    out = _unshard([res.results[i]["y"] for i in range(N_CORES)])
    kernel.last_results = res
    return out
